# revision 24
# baseline (speedup 1.0000x reference)
"""GaussianNB log-posterior kernel for 8 Trainium2 NeuronCores.

out[b, c] = log_pi[c] - 0.5 * sum_f(log2pi + log_var[c,f] + (x[b,f]-mu[c,f])^2 / var[c,f])
          = const_c + sum_f x^2[b,f]*wq[c,f] + sum_f x[b,f]*wc[c,f]
  with inv = exp(-lv), wq = -0.5*inv, wc = mu*inv,
       const_c = -0.5*(sum_f lv + sum_f mu*wc - 2*lp_c + F*log2pi)

Strategy: 4-way batch x 2-way class sharding (512 batch rows, 128 classes per
core) so each GEMM matmul streams N=512 (one full PSUM bank) and only 16
stationary loads are needed. Wire format fp16, f-major (host does layout
only: cast + transpose + pack; all arithmetic on device; fp8 wire was tried
and loses — DVE reads 1-byte dtypes at half rate, costing more than the DMA
saves). Weights travel as two packed chunks [lv_h0|mu_h0] and
[lv_h1|mu_h1|lp] so each 0.25MB chunk unlocks a full half-pipeline. Six
paced ~0.25MB DMA chunks (outstanding DMAs interleave packet-wise on the
shared engines, so chunk size must stay near the trigger spacing for
completions to arrive in order). The PE runs cheap warmup matmuls during the
DMA window and filler matmuls across known waits (the p-state ramp resets on
any idle gap and takes ~3us to return to full clock). out[c, b] =
sum_f wq/wc[c,f] * x2/x[b,f]; const_c is folded into the same PSUM
accumulation via a ones-column reduction stationary plus K=1 matmuls for lp
and F*log2pi, finished by a K=1 broadcast of -0.5*s.
Output (128c, 512b) fp16 per core, host reassembles and casts to f32.
"""
import sys

sys.path.insert(0, "/opt/trn_rl_repo")
import numpy as np
import concourse.bacc as bacc
import concourse.mybir as mybir
from concourse.tile import TileContext
from concourse.bass_utils import run_bass_kernel_spmd

B, C, F = 2048, 256, 1024
NCORES = 8
BSH = 512            # batch rows per core (4-way)
CSH = 128            # classes per core (2-way)
KT = F // 128        # 8 k-tiles
XW = KT * BSH        # 4096 packed x columns
WW = KT * CSH        # 1024 packed weight columns (per tensor)
WP = 2 * WW + CSH    # packed weight wire: lv | mu | lp
LOG_2PI = float(np.log(2.0 * np.pi))
F32 = mybir.dt.float32
F16 = mybir.dt.float16
AF = mybir.ActivationFunctionType
NWARM = 42

_CACHE = {}


def _build():
    nc = bacc.Bacc("TRN2", target_bir_lowering=False, debug=False, num_devices=NCORES)
    # w wire layout (128, 2176): [lv h0 | mu h0 | lv h1 | mu h1 | lp] where
    # each h is 512 packed cols (k-tiles 0-3 / 4-7); x packed (128, 4096).
    w_d = nc.dram_tensor("wt", [128, WP], F16, kind="ExternalInput").ap()
    x_d = nc.dram_tensor("xt", [128, XW], F16, kind="ExternalInput").ap()
    out_d = nc.dram_tensor("out", [128, BSH], F16, kind="ExternalOutput").ap()

    HW = WW // 2  # 512 cols per weight half
    with TileContext(nc) as tc:
        with (
            tc.tile_pool(name="sb", bufs=1) as sb,
            tc.tile_pool(name="po", bufs=1, space="PSUM") as po,
        ):
            wt = sb.tile([128, WP], F16, tag="wt")
            xt = sb.tile([128, XW], F16, tag="xt")
            xq = [slice(q * 1024, (q + 1) * 1024) for q in range(4)]  # 0.25MB each
            nc.sync.dma_start(out=wt[:, 0:2 * HW], in_=w_d[:, 0:2 * HW])
            nc.sync.dma_start(out=xt[:, xq[0]], in_=x_d[:, xq[0]])
            nc.sync.dma_start(out=wt[:, 2 * HW:WP], in_=w_d[:, 2 * HW:WP])
            nc.sync.dma_start(out=xt[:, xq[1]], in_=x_d[:, xq[1]])
            nc.sync.dma_start(out=xt[:, xq[2]], in_=x_d[:, xq[2]])
            nc.sync.dma_start(out=xt[:, xq[3]], in_=x_d[:, xq[3]])
            lvh = [wt[:, 0:HW], wt[:, 2 * HW:3 * HW]]
            muh = [wt[:, HW:2 * HW], wt[:, 3 * HW:4 * HW]]
            lp_row = wt[0:1, 4 * HW:4 * HW + CSH]

            # constants + PE warmup during the DMA window
            ones_col = sb.tile([128, 1], F16, tag="onc")
            neg2 = sb.tile([1, 1], F16, tag="n2")
            ones1 = sb.tile([1, 1], F16, tag="o1")
            dmy = sb.tile([1, 128], F16, tag="dmy")
            logc_row = sb.tile([1, CSH], F16, tag="logc")   # F*log2pi
            s_col = sb.tile([128, 1], F32, tag="scol")      # -0.5 * s per class
            nc.gpsimd.memset(ones_col[:], 1.0)
            nc.gpsimd.memset(neg2[:], -2.0)
            nc.gpsimd.memset(ones1[:], 1.0)
            nc.gpsimd.memset(dmy[:], 0.5)
            nc.gpsimd.memset(logc_row[:], F * LOG_2PI)
            tw = sb.tile([1, 1], F32, tag="tw")
            tw2 = sb.tile([1, 1], F32, tag="tw2")
            nc.gpsimd.memset(tw[:], 0.0)
            nc.scalar.activation(tw2[:], tw[:], AF.Exp)  # preload exp table
            wp = po.tile([128, 128], F32, tag="wp")
            for i in range(NWARM):
                nc.tensor.matmul(wp[:], dmy[:], dmy[:], start=True, stop=True)

            def fillers(n):
                for _ in range(n):
                    nc.tensor.matmul(wp[:], dmy[:], dmy[:], start=True, stop=True)

            # ---- prep (per half): inv = exp(-lv); wq = -0.5*inv; wc = mu*inv;
            # m2i = mu*wc; x2 = x*x per quarter
            invt = sb.tile([128, WW], F16, tag="invt")
            wqt = sb.tile([128, WW], F16, tag="wqt")
            wct = sb.tile([128, WW], F16, tag="wct")
            m2it = sb.tile([128, WW], F16, tag="m2it")
            x2t = sb.tile([128, XW], F16, tag="x2t")
            wh = [slice(0, HW), slice(HW, WW)]
            nc.scalar.activation(invt[:, wh[0]], lvh[0], AF.Exp, scale=-1.0)
            nc.scalar.activation(invt[:, wh[1]], lvh[1], AF.Exp, scale=-1.0)
            # DVE queue in expected readiness order
            nc.vector.tensor_mul(x2t[:, xq[0]], xt[:, xq[0]], xt[:, xq[0]])
            nc.vector.tensor_scalar_mul(wqt[:, wh[0]], invt[:, wh[0]], -0.5)
            nc.vector.tensor_mul(wct[:, wh[0]], muh[0], invt[:, wh[0]])
            nc.vector.tensor_mul(x2t[:, xq[1]], xt[:, xq[1]], xt[:, xq[1]])
            nc.vector.tensor_scalar_mul(wqt[:, wh[1]], invt[:, wh[1]], -0.5)
            nc.vector.tensor_mul(wct[:, wh[1]], muh[1], invt[:, wh[1]])
            nc.vector.tensor_mul(m2it[:, wh[0]], muh[0], wct[:, wh[0]])
            nc.vector.tensor_mul(x2t[:, xq[2]], xt[:, xq[2]], xt[:, xq[2]])
            nc.vector.tensor_mul(m2it[:, wh[1]], muh[1], wct[:, wh[1]])
            nc.vector.tensor_mul(x2t[:, xq[3]], xt[:, xq[3]], xt[:, xq[3]])

            # ---- GEMMs + folded const ----
            x3 = xt[:].rearrange("p (k n) -> p k n", k=KT)
            x23 = x2t[:].rearrange("p (k n) -> p k n", k=KT)
            wq3 = wqt[:].rearrange("p (k n) -> p k n", k=KT)
            wc3 = wct[:].rearrange("p (k n) -> p k n", k=KT)
            lv3l = [lvh[0].rearrange("p (k n) -> p k n", k=4),
                    lvh[1].rearrange("p (k n) -> p k n", k=4)]
            m23 = m2it[:].rearrange("p (k n) -> p k n", k=KT)
            pg = po.tile([128, BSH], F32, tag="pg")
            s_ps = po.tile([128, 1], F32, tag="sps")   # per-class s column
            step = [0]
            rstep = [0]
            NGEMM, NRED = 16, 18

            def gemms(W3, A3, ks):
                for k in ks:
                    step[0] += 1
                    nc.tensor.matmul(
                        pg[:], W3[:, k, :], A3[:, k, :],
                        start=(step[0] == 1), stop=(step[0] == NGEMM),
                        skip_group_check=True,
                    )

            def reds(T3, ks):
                # s[c] += sum_f T[f, c]: stationary = the tile, moving = ones
                for k in ks:
                    rstep[0] += 1
                    nc.tensor.matmul(
                        s_ps[:], T3[:, k, :], ones_col[:],
                        start=(rstep[0] == 1), stop=(rstep[0] == NRED),
                        skip_group_check=True,
                    )

            reds(lv3l[0], range(0, 4))         # ready with chunk w0, warms PE
            fillers(4)
            gemms(wq3, x23, range(0, 2))       # quad k0-1
            reds(lv3l[1], range(0, 4))         # ready with chunk w1
            gemms(wc3, x3, range(0, 4))        # cross h0
            gemms(wq3, x23, range(2, 4))       # quad k2-3
            reds(m23, range(0, 4))
            gemms(wc3, x3, range(4, 8))        # cross h1
            reds(m23, range(4, 8))
            # lp and F*log2pi folded into s: s += lp*(-2), s += log2pi_row*1
            rstep[0] += 1
            nc.tensor.matmul(s_ps[:], lp_row, neg2[:], start=False, stop=False,
                             skip_group_check=True)
            rstep[0] += 1
            nc.tensor.matmul(s_ps[:], logc_row[:], ones1[:], start=False,
                             stop=(rstep[0] == NRED), skip_group_check=True)
            nc.scalar.mul(s_col[:], s_ps[:], -0.5)   # ACT copy with scale
            gemms(wq3, x23, range(4, 6))       # quad k4-5
            gemms(wq3, x23, range(6, 8))       # quad k6-7 (stop)

            # ---- copy out (+ per-class const add) + DMA ----
            out_sb = sb.tile([128, BSH], F16, tag="osb")
            nc.vector.tensor_scalar_add(out_sb[:, 0:BSH // 2], pg[:, 0:BSH // 2],
                                        s_col[:])
            nc.vector.tensor_scalar_add(out_sb[:, BSH // 2:BSH],
                                        pg[:, BSH // 2:BSH], s_col[:])
            nc.sync.dma_start(out=out_d[:, :], in_=out_sb[:])

    nc.compile()
    return nc


def get_nc():
    if "nc" not in _CACHE:
        _CACHE["nc"] = _build()
    return _CACHE["nc"]


def _pack_fmajor(aT):
    # (F=1024, n) f-major -> SBUF-packed (128, 8*n): cols k*n..(k+1)*n = rows
    # k*128..(k+1)*128
    Fdim, n = aT.shape
    k = Fdim // 128
    return np.ascontiguousarray(
        aT.reshape(k, 128, n).transpose(1, 0, 2).reshape(128, k * n)
    )


def make_in_maps(x, mu, log_var, log_pi):
    x16 = np.asarray(x, dtype=np.float16)
    mu16 = np.asarray(mu, dtype=np.float16)
    lv16 = np.asarray(log_var, dtype=np.float16)
    lp16 = np.asarray(log_pi, dtype=np.float16)

    xT = x16.T                                      # (1024, 2048)
    muT = mu16.T                                    # (1024, 256)
    lvT = lv16.T
    HW = WW // 2
    maps = []
    for core in range(NCORES):
        bi, ci = divmod(core, 2)
        cs = slice(ci * CSH, (ci + 1) * CSH)
        lvp = _pack_fmajor(lvT[:, cs])              # (128, 1024)
        mup = _pack_fmajor(muT[:, cs])
        wt = np.zeros((128, WP), np.float16)
        wt[:, 0:HW] = lvp[:, 0:HW]
        wt[:, HW:2 * HW] = mup[:, 0:HW]
        wt[:, 2 * HW:3 * HW] = lvp[:, HW:WW]
        wt[:, 3 * HW:4 * HW] = mup[:, HW:WW]
        wt[0, 4 * HW:4 * HW + CSH] = lp16[cs]
        maps.append({
            "wt": np.ascontiguousarray(wt),
            "xt": _pack_fmajor(xT[:, bi * BSH:(bi + 1) * BSH]),
        })
    return maps


def unpack_out(res):
    out = np.empty((B, C), dtype=np.float32)
    for core in range(NCORES):
        bi, ci = divmod(core, 2)
        o = res.results[core]["out"]                # (128c, 512b) fp16
        out[bi * BSH:(bi + 1) * BSH, ci * CSH:(ci + 1) * CSH] = o.T
    return out


def kernel(x, mu, log_var, log_pi):
    nc = get_nc()
    in_maps = make_in_maps(x, mu, log_var, log_pi)
    res = run_bass_kernel_spmd(nc, in_maps, list(range(NCORES)))
    return unpack_out(res)


# revision 26
# speedup vs baseline: 1.0099x; 1.0099x over previous
"""GaussianNB log-posterior kernel for 8 Trainium2 NeuronCores.

out[b, c] = log_pi[c] - 0.5 * sum_f(log2pi + log_var[c,f] + (x[b,f]-mu[c,f])^2 / var[c,f])
          = const_c + sum_f x^2[b,f]*wq[c,f] + sum_f x[b,f]*wc[c,f]
  with inv = exp(-lv), wq = -0.5*inv, wc = mu*inv,
       const_c = -0.5*(sum_f lv + sum_f mu*wc - 2*lp_c + F*log2pi)

Strategy: 4-way batch x 2-way class sharding (512 batch rows, 128 classes per
core) so each GEMM matmul streams N=512 (one full PSUM bank) and only 16
stationary loads are needed. Wire format fp16, f-major (host does layout
only: cast + transpose + pack; all arithmetic on device; fp8 wire was tried
and loses — DVE reads 1-byte dtypes at half rate, costing more than the DMA
saves). Weights travel as two packed chunks [lv_h0|mu_h0] and
[lv_h1|mu_h1|lp] so each 0.25MB chunk unlocks a full half-pipeline. Six
paced ~0.25MB DMA chunks (outstanding DMAs interleave packet-wise on the
shared engines, so chunk size must stay near the trigger spacing for
completions to arrive in order). The PE runs cheap warmup matmuls during the
DMA window and filler matmuls across known waits (the p-state ramp resets on
any idle gap and takes ~3us to return to full clock). out[c, b] =
sum_f wq/wc[c,f] * x2/x[b,f]; const_c is folded into the same PSUM
accumulation via a ones-column reduction stationary plus K=1 matmuls for lp
and F*log2pi, finished by a K=1 broadcast of -0.5*s.
Output (128c, 512b) fp16 per core, host reassembles and casts to f32.
"""
import sys

sys.path.insert(0, "/opt/trn_rl_repo")
import numpy as np
import concourse.bacc as bacc
import concourse.mybir as mybir
from concourse.tile import TileContext
from concourse.bass_utils import run_bass_kernel_spmd

B, C, F = 2048, 256, 1024
NCORES = 8
BSH = 512            # batch rows per core (4-way)
CSH = 128            # classes per core (2-way)
KT = F // 128        # 8 k-tiles
XW = KT * BSH        # 4096 packed x columns
WW = KT * CSH        # 1024 packed weight columns (per tensor)
WP = 2 * WW + CSH    # packed weight wire: lv | mu | lp
LOG_2PI = float(np.log(2.0 * np.pi))
F32 = mybir.dt.float32
F16 = mybir.dt.float16
AF = mybir.ActivationFunctionType
NWARM = 42

_CACHE = {}


def _build():
    nc = bacc.Bacc("TRN2", target_bir_lowering=False, debug=False, num_devices=NCORES)
    # w wire layout (128, 2176): [lv h0 | mu h0 | lv h1 | mu h1 | lp] where
    # each h is 512 packed cols (k-tiles 0-3 / 4-7); x packed (128, 4096).
    w_d = nc.dram_tensor("wt", [128, WP], F16, kind="ExternalInput").ap()
    x_d = nc.dram_tensor("xt", [128, XW], F16, kind="ExternalInput").ap()
    out_d = nc.dram_tensor("out", [128, BSH], F16, kind="ExternalOutput").ap()

    HW = WW // 2  # 512 cols per weight half
    with TileContext(nc) as tc:
        with (
            tc.tile_pool(name="sb", bufs=1) as sb,
            tc.tile_pool(name="po", bufs=1, space="PSUM") as po,
        ):
            wt = sb.tile([128, WP], F16, tag="wt")
            xt = sb.tile([128, XW], F16, tag="xt")
            xq = [slice(q * 1024, (q + 1) * 1024) for q in range(4)]  # 0.25MB each
            nc.sync.dma_start(out=wt[:, 0:2 * HW], in_=w_d[:, 0:2 * HW])
            nc.sync.dma_start(out=xt[:, xq[0]], in_=x_d[:, xq[0]])
            nc.sync.dma_start(out=wt[:, 2 * HW:WP], in_=w_d[:, 2 * HW:WP])
            nc.sync.dma_start(out=xt[:, xq[1]], in_=x_d[:, xq[1]])
            nc.sync.dma_start(out=xt[:, xq[2]], in_=x_d[:, xq[2]])
            nc.sync.dma_start(out=xt[:, xq[3]], in_=x_d[:, xq[3]])
            lvh = [wt[:, 0:HW], wt[:, 2 * HW:3 * HW]]
            muh = [wt[:, HW:2 * HW], wt[:, 3 * HW:4 * HW]]
            lp_row = wt[0:1, 4 * HW:4 * HW + CSH]

            # constants + PE warmup during the DMA window
            ones_col = sb.tile([128, 1], F16, tag="onc")
            neg2 = sb.tile([1, 1], F16, tag="n2")
            ones1 = sb.tile([1, 1], F16, tag="o1")
            dmy = sb.tile([1, 128], F16, tag="dmy")
            logc_row = sb.tile([1, CSH], F16, tag="logc")   # F*log2pi
            s_col = sb.tile([128, 1], F32, tag="scol")      # -0.5 * s per class
            nc.gpsimd.memset(ones_col[:], 1.0)
            nc.gpsimd.memset(neg2[:], -2.0)
            nc.gpsimd.memset(ones1[:], 1.0)
            nc.gpsimd.memset(dmy[:], 0.5)
            nc.gpsimd.memset(logc_row[:], F * LOG_2PI)
            tw = sb.tile([1, 1], F32, tag="tw")
            tw2 = sb.tile([1, 1], F32, tag="tw2")
            nc.gpsimd.memset(tw[:], 0.0)
            nc.scalar.activation(tw2[:], tw[:], AF.Exp)  # preload exp table
            wp = po.tile([128, 128], F32, tag="wp")
            for i in range(NWARM):
                nc.tensor.matmul(wp[:], dmy[:], dmy[:], start=True, stop=True)

            def fillers(n):
                for _ in range(n):
                    nc.tensor.matmul(wp[:], dmy[:], dmy[:], start=True, stop=True)

            # ---- prep (per half): inv = exp(-lv); wq = -0.5*inv; wc = mu*inv;
            # m2i = mu*wc; x2 = x*x per quarter
            invt = sb.tile([128, WW], F16, tag="invt")
            wqt = sb.tile([128, WW], F16, tag="wqt")
            wct = sb.tile([128, WW], F16, tag="wct")
            m2it = sb.tile([128, WW], F16, tag="m2it")
            x2t = sb.tile([128, XW], F16, tag="x2t")
            wh = [slice(0, HW), slice(HW, WW)]
            nc.scalar.activation(invt[:, wh[0]], lvh[0], AF.Exp, scale=-1.0)
            nc.scalar.activation(invt[:, wh[1]], lvh[1], AF.Exp, scale=-1.0)
            # DVE queue in expected readiness order
            nc.vector.tensor_mul(x2t[:, xq[0]], xt[:, xq[0]], xt[:, xq[0]])
            nc.vector.tensor_scalar_mul(wqt[:, wh[0]], invt[:, wh[0]], -0.5)
            nc.vector.tensor_mul(wct[:, wh[0]], muh[0], invt[:, wh[0]])
            nc.vector.tensor_mul(x2t[:, xq[1]], xt[:, xq[1]], xt[:, xq[1]])
            nc.vector.tensor_scalar_mul(wqt[:, wh[1]], invt[:, wh[1]], -0.5)
            nc.vector.tensor_mul(wct[:, wh[1]], muh[1], invt[:, wh[1]])
            nc.vector.tensor_mul(m2it[:, wh[0]], muh[0], wct[:, wh[0]])
            nc.vector.tensor_mul(x2t[:, xq[2]], xt[:, xq[2]], xt[:, xq[2]])
            nc.vector.tensor_mul(m2it[:, wh[1]], muh[1], wct[:, wh[1]])
            nc.vector.tensor_mul(x2t[:, xq[3]], xt[:, xq[3]], xt[:, xq[3]])

            # ---- GEMMs + folded const ----
            x3 = xt[:].rearrange("p (k n) -> p k n", k=KT)
            x23 = x2t[:].rearrange("p (k n) -> p k n", k=KT)
            wq3 = wqt[:].rearrange("p (k n) -> p k n", k=KT)
            wc3 = wct[:].rearrange("p (k n) -> p k n", k=KT)
            lv3l = [lvh[0].rearrange("p (k n) -> p k n", k=4),
                    lvh[1].rearrange("p (k n) -> p k n", k=4)]
            m23 = m2it[:].rearrange("p (k n) -> p k n", k=KT)
            pg = po.tile([128, BSH], F32, tag="pg")
            s_ps = po.tile([128, 1], F32, tag="sps")   # per-class s column
            step = [0]
            rstep = [0]
            NGEMM, NRED = 16, 18

            def gemms(W3, A3, ks):
                for k in ks:
                    step[0] += 1
                    nc.tensor.matmul(
                        pg[:], W3[:, k, :], A3[:, k, :],
                        start=(step[0] == 1), stop=(step[0] == NGEMM),
                        skip_group_check=True,
                    )

            def reds(T3, ks):
                # s[c] += sum_f T[f, c]: stationary = the tile, moving = ones
                for k in ks:
                    rstep[0] += 1
                    nc.tensor.matmul(
                        s_ps[:], T3[:, k, :], ones_col[:],
                        start=(rstep[0] == 1), stop=(rstep[0] == NRED),
                        skip_group_check=True,
                    )

            reds(lv3l[0], range(0, 4))         # ready with chunk w0, warms PE
            fillers(7)
            gemms(wq3, x23, range(0, 2))       # quad k0-1
            reds(lv3l[1], range(0, 4))         # ready with chunk w1
            gemms(wc3, x3, range(0, 4))        # cross h0
            gemms(wq3, x23, range(2, 4))       # quad k2-3
            reds(m23, range(0, 4))
            gemms(wc3, x3, range(4, 8))        # cross h1
            reds(m23, range(4, 8))
            # lp and F*log2pi folded into s: s += lp*(-2), s += log2pi_row*1
            rstep[0] += 1
            nc.tensor.matmul(s_ps[:], lp_row, neg2[:], start=False, stop=False,
                             skip_group_check=True)
            rstep[0] += 1
            nc.tensor.matmul(s_ps[:], logc_row[:], ones1[:], start=False,
                             stop=(rstep[0] == NRED), skip_group_check=True)
            nc.scalar.mul(s_col[:], s_ps[:], -0.5)   # ACT copy with scale
            gemms(wq3, x23, range(4, 6))       # quad k4-5
            gemms(wq3, x23, range(6, 8))       # quad k6-7 (stop)

            # ---- copy out (+ per-class const add) + DMA ----
            out_sb = sb.tile([128, BSH], F16, tag="osb")
            nc.vector.tensor_scalar_add(out_sb[:], pg[:], s_col[:])
            nc.sync.dma_start(out=out_d[:, :], in_=out_sb[:])

    nc.compile()
    return nc


def get_nc():
    if "nc" not in _CACHE:
        _CACHE["nc"] = _build()
    return _CACHE["nc"]


def _pack_fmajor(aT):
    # (F=1024, n) f-major -> SBUF-packed (128, 8*n): cols k*n..(k+1)*n = rows
    # k*128..(k+1)*128
    Fdim, n = aT.shape
    k = Fdim // 128
    return np.ascontiguousarray(
        aT.reshape(k, 128, n).transpose(1, 0, 2).reshape(128, k * n)
    )


def make_in_maps(x, mu, log_var, log_pi):
    x16 = np.asarray(x, dtype=np.float16)
    mu16 = np.asarray(mu, dtype=np.float16)
    lv16 = np.asarray(log_var, dtype=np.float16)
    lp16 = np.asarray(log_pi, dtype=np.float16)

    xT = x16.T                                      # (1024, 2048)
    muT = mu16.T                                    # (1024, 256)
    lvT = lv16.T
    HW = WW // 2
    maps = []
    for core in range(NCORES):
        bi, ci = divmod(core, 2)
        cs = slice(ci * CSH, (ci + 1) * CSH)
        lvp = _pack_fmajor(lvT[:, cs])              # (128, 1024)
        mup = _pack_fmajor(muT[:, cs])
        wt = np.zeros((128, WP), np.float16)
        wt[:, 0:HW] = lvp[:, 0:HW]
        wt[:, HW:2 * HW] = mup[:, 0:HW]
        wt[:, 2 * HW:3 * HW] = lvp[:, HW:WW]
        wt[:, 3 * HW:4 * HW] = mup[:, HW:WW]
        wt[0, 4 * HW:4 * HW + CSH] = lp16[cs]
        maps.append({
            "wt": np.ascontiguousarray(wt),
            "xt": _pack_fmajor(xT[:, bi * BSH:(bi + 1) * BSH]),
        })
    return maps


def unpack_out(res):
    out = np.empty((B, C), dtype=np.float32)
    for core in range(NCORES):
        bi, ci = divmod(core, 2)
        o = res.results[core]["out"]                # (128c, 512b) fp16
        out[bi * BSH:(bi + 1) * BSH, ci * CSH:(ci + 1) * CSH] = o.T
    return out


def kernel(x, mu, log_var, log_pi):
    nc = get_nc()
    in_maps = make_in_maps(x, mu, log_var, log_pi)
    res = run_bass_kernel_spmd(nc, in_maps, list(range(NCORES)))
    return unpack_out(res)


# revision 27
# speedup vs baseline: 1.0530x; 1.0427x over previous
"""GaussianNB log-posterior kernel for 8 Trainium2 NeuronCores.

out[b, c] = log_pi[c] - 0.5 * sum_f(log2pi + log_var[c,f] + (x[b,f]-mu[c,f])^2 / var[c,f])
          = const_c + sum_f x^2[b,f]*wq[c,f] + sum_f x[b,f]*wc[c,f]
  with inv = exp(-lv), wq = -0.5*inv, wc = mu*inv,
       const_c = -0.5*(sum_f lv + sum_f mu*wc - 2*lp_c + F*log2pi)

Strategy: 4-way batch x 2-way class sharding (512 batch rows, 128 classes per
core) so each GEMM matmul streams N=512 (one full PSUM bank) and only 16
stationary loads are needed. Wire format fp16, f-major (host does layout
only: cast + transpose + pack; all arithmetic on device; fp8 wire was tried
and loses — DVE reads 1-byte dtypes at half rate, costing more than the DMA
saves). Weights travel as two packed chunks [lv_h0|mu_h0] and
[lv_h1|mu_h1|lp] so each 0.25MB chunk unlocks a full half-pipeline. Six
paced ~0.25MB DMA chunks (outstanding DMAs interleave packet-wise on the
shared engines, so chunk size must stay near the trigger spacing for
completions to arrive in order). The PE runs cheap warmup matmuls during the
DMA window and filler matmuls across known waits (the p-state ramp resets on
any idle gap and takes ~3us to return to full clock). out[c, b] =
sum_f wq/wc[c,f] * x2/x[b,f]. const_c is a per-partition (per-class) scalar
here: the sum_f reductions accumulate a (128,1) s column on the PE (tile
stationary x ones moving, plus K=1 matmuls folding lp and F*log2pi), and the
copyout adds -0.5*s as a tensor_scalar bias while casting PSUM to fp16.
Output (128c, 512b) fp16 per core, host reassembles and casts to f32.
"""
import sys

sys.path.insert(0, "/opt/trn_rl_repo")
import numpy as np
import concourse.bacc as bacc
import concourse.mybir as mybir
from concourse.tile import TileContext
from concourse.bass_utils import run_bass_kernel_spmd

B, C, F = 2048, 256, 1024
NCORES = 8
BSH = 512            # batch rows per core (4-way)
CSH = 128            # classes per core (2-way)
KT = F // 128        # 8 k-tiles
XW = KT * BSH        # 4096 packed x columns
WW = KT * CSH        # 1024 packed weight columns (per tensor)
WP = 2 * WW + CSH    # packed weight wire: lv | mu | lp
LOG_2PI = float(np.log(2.0 * np.pi))
F32 = mybir.dt.float32
F16 = mybir.dt.float16
AF = mybir.ActivationFunctionType
NWARM = 42

_CACHE = {}


def _build():
    nc = bacc.Bacc("TRN2", target_bir_lowering=False, debug=False, num_devices=NCORES)
    # w wire layout (128, 2176): [lv h0 | mu h0 | lv h1 | mu h1 | lp] where
    # each h is 512 packed cols (k-tiles 0-3 / 4-7); x packed (128, 4096).
    w_d = nc.dram_tensor("wt", [128, WP], F16, kind="ExternalInput").ap()
    x_d = nc.dram_tensor("xt", [128, XW], F16, kind="ExternalInput").ap()
    out_d = nc.dram_tensor("out", [128, BSH], F16, kind="ExternalOutput").ap()

    HW = WW // 2  # 512 cols per weight half
    with TileContext(nc) as tc:
        with (
            tc.tile_pool(name="sb", bufs=1) as sb,
            tc.tile_pool(name="po", bufs=1, space="PSUM") as po,
        ):
            wt = sb.tile([128, WP], F16, tag="wt")
            xt = sb.tile([128, XW], F16, tag="xt")
            xq = [slice(q * 1024, (q + 1) * 1024) for q in range(4)]  # 0.25MB each
            nc.sync.dma_start(out=wt[:, 0:2 * HW], in_=w_d[:, 0:2 * HW])
            nc.sync.dma_start(out=xt[:, xq[0]], in_=x_d[:, xq[0]])
            nc.sync.dma_start(out=wt[:, 2 * HW:WP], in_=w_d[:, 2 * HW:WP])
            nc.sync.dma_start(out=xt[:, xq[1]], in_=x_d[:, xq[1]])
            nc.sync.dma_start(out=xt[:, xq[2]], in_=x_d[:, xq[2]])
            nc.sync.dma_start(out=xt[:, xq[3]], in_=x_d[:, xq[3]])
            lvh = [wt[:, 0:HW], wt[:, 2 * HW:3 * HW]]
            muh = [wt[:, HW:2 * HW], wt[:, 3 * HW:4 * HW]]
            lp_row = wt[0:1, 4 * HW:4 * HW + CSH]

            # constants + PE warmup during the DMA window
            ones_col = sb.tile([128, 1], F16, tag="onc")
            neg2 = sb.tile([1, 1], F16, tag="n2")
            ones1 = sb.tile([1, 1], F16, tag="o1")
            dmy = sb.tile([1, 128], F16, tag="dmy")
            logc_row = sb.tile([1, CSH], F16, tag="logc")   # F*log2pi
            s_col = sb.tile([128, 1], F32, tag="scol")      # -0.5 * s per class
            nc.gpsimd.memset(ones_col[:], 1.0)
            nc.gpsimd.memset(neg2[:], -2.0)
            nc.gpsimd.memset(ones1[:], 1.0)
            nc.gpsimd.memset(dmy[:], 0.5)
            nc.gpsimd.memset(logc_row[:], F * LOG_2PI)
            tw = sb.tile([1, 1], F32, tag="tw")
            tw2 = sb.tile([1, 1], F32, tag="tw2")
            nc.gpsimd.memset(tw[:], 0.0)
            nc.scalar.activation(tw2[:], tw[:], AF.Exp)  # preload exp table
            wp = po.tile([128, 128], F32, tag="wp")
            for i in range(NWARM):
                nc.tensor.matmul(wp[:], dmy[:], dmy[:], start=True, stop=True)

            def fillers(n):
                for _ in range(n):
                    nc.tensor.matmul(wp[:], dmy[:], dmy[:], start=True, stop=True)

            # ---- prep (per half): inv = exp(-lv); wq = -0.5*inv; wc = mu*inv;
            # m2i = mu*wc; x2 = x*x per quarter
            invt = sb.tile([128, WW], F16, tag="invt")
            wqt = sb.tile([128, WW], F16, tag="wqt")
            wct = sb.tile([128, WW], F16, tag="wct")
            m2it = sb.tile([128, WW], F16, tag="m2it")
            x2t = sb.tile([128, XW], F16, tag="x2t")
            wh = [slice(0, HW), slice(HW, WW)]
            nc.scalar.activation(invt[:, wh[0]], lvh[0], AF.Exp, scale=-1.0)
            nc.scalar.activation(invt[:, wh[1]], lvh[1], AF.Exp, scale=-1.0)
            # DVE queue in expected readiness order
            nc.vector.tensor_mul(x2t[:, xq[0]], xt[:, xq[0]], xt[:, xq[0]])
            nc.vector.tensor_scalar_mul(wqt[:, wh[0]], invt[:, wh[0]], -0.5)
            nc.vector.tensor_mul(wct[:, wh[0]], muh[0], invt[:, wh[0]])
            nc.vector.tensor_mul(x2t[:, xq[1]], xt[:, xq[1]], xt[:, xq[1]])
            nc.vector.tensor_scalar_mul(wqt[:, wh[1]], invt[:, wh[1]], -0.5)
            nc.vector.tensor_mul(wct[:, wh[1]], muh[1], invt[:, wh[1]])
            nc.vector.tensor_mul(m2it[:, wh[0]], muh[0], wct[:, wh[0]])
            nc.vector.tensor_mul(x2t[:, xq[2]], xt[:, xq[2]], xt[:, xq[2]])
            nc.vector.tensor_mul(m2it[:, wh[1]], muh[1], wct[:, wh[1]])
            nc.vector.tensor_mul(x2t[:, xq[3]], xt[:, xq[3]], xt[:, xq[3]])

            # ---- GEMMs + folded const ----
            x3 = xt[:].rearrange("p (k n) -> p k n", k=KT)
            x23 = x2t[:].rearrange("p (k n) -> p k n", k=KT)
            wq3 = wqt[:].rearrange("p (k n) -> p k n", k=KT)
            wc3 = wct[:].rearrange("p (k n) -> p k n", k=KT)
            lv3l = [lvh[0].rearrange("p (k n) -> p k n", k=4),
                    lvh[1].rearrange("p (k n) -> p k n", k=4)]
            m23 = m2it[:].rearrange("p (k n) -> p k n", k=KT)
            pg = po.tile([128, BSH], F32, tag="pg")
            s_ps = po.tile([128, 1], F32, tag="sps")   # per-class s column
            step = [0]
            rstep = [0]
            NGEMM, NRED = 16, 18

            def gemms(W3, A3, ks):
                for k in ks:
                    step[0] += 1
                    nc.tensor.matmul(
                        pg[:], W3[:, k, :], A3[:, k, :],
                        start=(step[0] == 1), stop=(step[0] == NGEMM),
                        skip_group_check=True,
                    )

            def reds(T3, ks):
                # s[c] += sum_f T[f, c]: stationary = the tile, moving = ones
                for k in ks:
                    rstep[0] += 1
                    nc.tensor.matmul(
                        s_ps[:], T3[:, k, :], ones_col[:],
                        start=(rstep[0] == 1), stop=(rstep[0] == NRED),
                        skip_group_check=True,
                    )

            reds(lv3l[0], range(0, 4))         # ready with chunk w0, warms PE
            fillers(7)
            gemms(wq3, x23, range(0, 2))       # quad k0-1
            reds(lv3l[1], range(0, 4))         # ready with chunk w1
            gemms(wc3, x3, range(0, 4))        # cross h0
            gemms(wq3, x23, range(2, 4))       # quad k2-3
            reds(m23, range(0, 4))
            gemms(wc3, x3, range(4, 8))        # cross h1
            reds(m23, range(4, 8))
            # lp and F*log2pi folded into s: s += lp*(-2), s += log2pi_row*1
            rstep[0] += 1
            nc.tensor.matmul(s_ps[:], lp_row, neg2[:], start=False, stop=False,
                             skip_group_check=True)
            rstep[0] += 1
            nc.tensor.matmul(s_ps[:], logc_row[:], ones1[:], start=False,
                             stop=(rstep[0] == NRED), skip_group_check=True)
            nc.scalar.mul(s_col[:], s_ps[:], -0.5)   # ACT copy with scale
            gemms(wq3, x23, range(4, 6))       # quad k4-5
            gemms(wq3, x23, range(6, 8))       # quad k6-7 (stop)

            # ---- copy out (+ per-class const add) + DMA ----
            out_sb = sb.tile([128, BSH], F16, tag="osb")
            nc.vector.tensor_scalar_add(out_sb[:], pg[:], s_col[:])
            nc.sync.dma_start(out=out_d[:, :], in_=out_sb[:])

    nc.compile()
    return nc


def get_nc():
    if "nc" not in _CACHE:
        _CACHE["nc"] = _build()
    return _CACHE["nc"]


def _pack_fmajor(aT):
    # (F=1024, n) f-major -> SBUF-packed (128, 8*n): cols k*n..(k+1)*n = rows
    # k*128..(k+1)*128
    Fdim, n = aT.shape
    k = Fdim // 128
    return np.ascontiguousarray(
        aT.reshape(k, 128, n).transpose(1, 0, 2).reshape(128, k * n)
    )


def make_in_maps(x, mu, log_var, log_pi):
    x16 = np.asarray(x, dtype=np.float16)
    mu16 = np.asarray(mu, dtype=np.float16)
    lv16 = np.asarray(log_var, dtype=np.float16)
    lp16 = np.asarray(log_pi, dtype=np.float16)

    xT = x16.T                                      # (1024, 2048)
    muT = mu16.T                                    # (1024, 256)
    lvT = lv16.T
    HW = WW // 2
    maps = []
    for core in range(NCORES):
        bi, ci = divmod(core, 2)
        cs = slice(ci * CSH, (ci + 1) * CSH)
        lvp = _pack_fmajor(lvT[:, cs])              # (128, 1024)
        mup = _pack_fmajor(muT[:, cs])
        wt = np.zeros((128, WP), np.float16)
        wt[:, 0:HW] = lvp[:, 0:HW]
        wt[:, HW:2 * HW] = mup[:, 0:HW]
        wt[:, 2 * HW:3 * HW] = lvp[:, HW:WW]
        wt[:, 3 * HW:4 * HW] = mup[:, HW:WW]
        wt[0, 4 * HW:4 * HW + CSH] = lp16[cs]
        maps.append({
            "wt": np.ascontiguousarray(wt),
            "xt": _pack_fmajor(xT[:, bi * BSH:(bi + 1) * BSH]),
        })
    return maps


def unpack_out(res):
    out = np.empty((B, C), dtype=np.float32)
    for core in range(NCORES):
        bi, ci = divmod(core, 2)
        o = res.results[core]["out"]                # (128c, 512b) fp16
        out[bi * BSH:(bi + 1) * BSH, ci * CSH:(ci + 1) * CSH] = o.T
    return out


def kernel(x, mu, log_var, log_pi):
    nc = get_nc()
    in_maps = make_in_maps(x, mu, log_var, log_pi)
    res = run_bass_kernel_spmd(nc, in_maps, list(range(NCORES)))
    return unpack_out(res)


# revision 28
# speedup vs baseline: 1.0538x; 1.0008x over previous
"""GaussianNB log-posterior kernel for 8 Trainium2 NeuronCores.

out[b, c] = log_pi[c] - 0.5 * sum_f(log2pi + log_var[c,f] + (x[b,f]-mu[c,f])^2 / var[c,f])
          = const_c + sum_f x^2[b,f]*wq[c,f] + sum_f x[b,f]*wc[c,f]
  with inv = exp(-lv), wq = -0.5*inv, wc = mu*inv,
       const_c = -0.5*(sum_f lv + sum_f mu*wc - 2*lp_c + F*log2pi)

Strategy: 4-way batch x 2-way class sharding (512 batch rows, 128 classes per
core) so each GEMM matmul streams N=512 (one full PSUM bank) and only 16
stationary loads are needed. Wire format fp16, f-major (host does layout
only: cast + transpose + pack; all arithmetic on device; fp8 wire was tried
and loses — DVE reads 1-byte dtypes at half rate, costing more than the DMA
saves). Weights travel as two packed chunks [lv_h0|mu_h0] and
[lv_h1|mu_h1|lp] so each 0.25MB chunk unlocks a full half-pipeline. Six
paced ~0.25MB DMA chunks (outstanding DMAs interleave packet-wise on the
shared engines, so chunk size must stay near the trigger spacing for
completions to arrive in order). The PE runs cheap warmup matmuls during the
DMA window and filler matmuls across known waits (the p-state ramp resets on
any idle gap and takes ~3us to return to full clock). out[c, b] =
sum_f wq/wc[c,f] * x2/x[b,f]. const_c is a per-partition (per-class) scalar
here: the sum_f reductions accumulate a (128,1) s column on the PE (tile
stationary x ones moving, plus K=1 matmuls folding lp and F*log2pi), and the
copyout adds -0.5*s as a tensor_scalar bias while casting PSUM to fp16.
Output (128c, 512b) fp16 per core, host reassembles and casts to f32.
"""
import sys

sys.path.insert(0, "/opt/trn_rl_repo")
import numpy as np
import concourse.bacc as bacc
import concourse.mybir as mybir
from concourse.tile import TileContext
from concourse.bass_utils import run_bass_kernel_spmd

B, C, F = 2048, 256, 1024
NCORES = 8
BSH = 512            # batch rows per core (4-way)
CSH = 128            # classes per core (2-way)
KT = F // 128        # 8 k-tiles
XW = KT * BSH        # 4096 packed x columns
WW = KT * CSH        # 1024 packed weight columns (per tensor)
WP = 2 * WW + CSH    # packed weight wire: lv | mu | lp
LOG_2PI = float(np.log(2.0 * np.pi))
F32 = mybir.dt.float32
F16 = mybir.dt.float16
AF = mybir.ActivationFunctionType
NWARM = 42

_CACHE = {}


def _build():
    nc = bacc.Bacc("TRN2", target_bir_lowering=False, debug=False, num_devices=NCORES)
    # w wire layout (128, 2176): [lv h0 | mu h0 | lv h1 | mu h1 | lp] where
    # each h is 512 packed cols (k-tiles 0-3 / 4-7); x packed (128, 4096).
    w_d = nc.dram_tensor("wt", [128, WP], F16, kind="ExternalInput").ap()
    x_d = nc.dram_tensor("xt", [128, XW], F16, kind="ExternalInput").ap()
    out_d = nc.dram_tensor("out", [128, BSH], F16, kind="ExternalOutput").ap()

    HW = WW // 2  # 512 cols per weight half
    with TileContext(nc) as tc:
        with (
            tc.tile_pool(name="sb", bufs=1) as sb,
            tc.tile_pool(name="po", bufs=1, space="PSUM") as po,
        ):
            wt = sb.tile([128, WP], F16, tag="wt")
            xt = sb.tile([128, XW], F16, tag="xt")
            xq = [slice(q * 1024, (q + 1) * 1024) for q in range(4)]  # 0.25MB each
            nc.sync.dma_start(out=wt[:, 0:2 * HW], in_=w_d[:, 0:2 * HW])
            nc.sync.dma_start(out=xt[:, xq[0]], in_=x_d[:, xq[0]])
            nc.sync.dma_start(out=wt[:, 2 * HW:WP], in_=w_d[:, 2 * HW:WP])
            nc.sync.dma_start(out=xt[:, xq[1]], in_=x_d[:, xq[1]])
            nc.sync.dma_start(out=xt[:, xq[2]], in_=x_d[:, xq[2]])
            nc.sync.dma_start(out=xt[:, xq[3]], in_=x_d[:, xq[3]])
            lvh = [wt[:, 0:HW], wt[:, 2 * HW:3 * HW]]
            muh = [wt[:, HW:2 * HW], wt[:, 3 * HW:4 * HW]]
            lp_row = wt[0:1, 4 * HW:4 * HW + CSH]

            # constants + PE warmup during the DMA window
            ones_col = sb.tile([128, 1], F16, tag="onc")
            neg2 = sb.tile([1, 1], F16, tag="n2")
            ones1 = sb.tile([1, 1], F16, tag="o1")
            dmy = sb.tile([1, 128], F16, tag="dmy")
            logc_row = sb.tile([1, CSH], F16, tag="logc")   # F*log2pi
            s_col = sb.tile([128, 1], F32, tag="scol")      # -0.5 * s per class
            nc.gpsimd.memset(ones_col[:], 1.0)
            nc.gpsimd.memset(neg2[:], -2.0)
            nc.gpsimd.memset(ones1[:], 1.0)
            nc.gpsimd.memset(dmy[:], 0.5)
            nc.gpsimd.memset(logc_row[:], F * LOG_2PI)
            tw = sb.tile([1, 1], F32, tag="tw")
            tw2 = sb.tile([1, 1], F32, tag="tw2")
            nc.gpsimd.memset(tw[:], 0.0)
            nc.scalar.activation(tw2[:], tw[:], AF.Exp)  # preload exp table
            wp = po.tile([128, 128], F32, tag="wp")
            for i in range(NWARM):
                nc.tensor.matmul(wp[:], dmy[:], dmy[:], start=True, stop=True)

            def fillers(n):
                for _ in range(n):
                    nc.tensor.matmul(wp[:], dmy[:], dmy[:], start=True, stop=True)

            # ---- prep (per half): inv = exp(-lv); wq = -0.5*inv; wc = mu*inv;
            # m2i = mu*wc; x2 = x*x per quarter
            invt = sb.tile([128, WW], F16, tag="invt")
            wqt = sb.tile([128, WW], F16, tag="wqt")
            wct = sb.tile([128, WW], F16, tag="wct")
            m2it = sb.tile([128, WW], F16, tag="m2it")
            x2t = sb.tile([128, XW], F16, tag="x2t")
            wh = [slice(0, HW), slice(HW, WW)]
            nc.scalar.activation(invt[:, wh[0]], lvh[0], AF.Exp, scale=-1.0)
            nc.scalar.activation(invt[:, wh[1]], lvh[1], AF.Exp, scale=-1.0)
            # DVE queue in expected readiness order
            nc.vector.tensor_mul(x2t[:, xq[0]], xt[:, xq[0]], xt[:, xq[0]])
            nc.vector.tensor_scalar_mul(wqt[:, wh[0]], invt[:, wh[0]], -0.5)
            nc.vector.tensor_mul(wct[:, wh[0]], muh[0], invt[:, wh[0]])
            nc.vector.tensor_mul(x2t[:, xq[1]], xt[:, xq[1]], xt[:, xq[1]])
            nc.vector.tensor_scalar_mul(wqt[:, wh[1]], invt[:, wh[1]], -0.5)
            nc.vector.tensor_mul(wct[:, wh[1]], muh[1], invt[:, wh[1]])
            nc.gpsimd.tensor_mul(m2it[:, wh[0]], muh[0], wct[:, wh[0]])
            nc.vector.tensor_mul(x2t[:, xq[2]], xt[:, xq[2]], xt[:, xq[2]])
            nc.gpsimd.tensor_mul(m2it[:, wh[1]], muh[1], wct[:, wh[1]])
            nc.vector.tensor_mul(x2t[:, xq[3]], xt[:, xq[3]], xt[:, xq[3]])

            # ---- GEMMs + folded const ----
            x3 = xt[:].rearrange("p (k n) -> p k n", k=KT)
            x23 = x2t[:].rearrange("p (k n) -> p k n", k=KT)
            wq3 = wqt[:].rearrange("p (k n) -> p k n", k=KT)
            wc3 = wct[:].rearrange("p (k n) -> p k n", k=KT)
            lv3l = [lvh[0].rearrange("p (k n) -> p k n", k=4),
                    lvh[1].rearrange("p (k n) -> p k n", k=4)]
            m23 = m2it[:].rearrange("p (k n) -> p k n", k=KT)
            pg = po.tile([128, BSH], F32, tag="pg")
            s_ps = po.tile([128, 1], F32, tag="sps")   # per-class s column
            step = [0]
            rstep = [0]
            NGEMM, NRED = 16, 18

            def gemms(W3, A3, ks):
                for k in ks:
                    step[0] += 1
                    nc.tensor.matmul(
                        pg[:], W3[:, k, :], A3[:, k, :],
                        start=(step[0] == 1), stop=(step[0] == NGEMM),
                        skip_group_check=True,
                    )

            def reds(T3, ks):
                # s[c] += sum_f T[f, c]: stationary = the tile, moving = ones
                for k in ks:
                    rstep[0] += 1
                    nc.tensor.matmul(
                        s_ps[:], T3[:, k, :], ones_col[:],
                        start=(rstep[0] == 1), stop=(rstep[0] == NRED),
                        skip_group_check=True,
                    )

            reds(lv3l[0], range(0, 4))         # ready with chunk w0, warms PE
            fillers(7)
            gemms(wq3, x23, range(0, 2))       # quad k0-1
            reds(lv3l[1], range(0, 4))         # ready with chunk w1
            gemms(wc3, x3, range(0, 4))        # cross h0
            gemms(wq3, x23, range(2, 4))       # quad k2-3
            reds(m23, range(0, 4))
            gemms(wc3, x3, range(4, 8))        # cross h1
            reds(m23, range(4, 8))
            # lp and F*log2pi folded into s: s += lp*(-2), s += log2pi_row*1
            rstep[0] += 1
            nc.tensor.matmul(s_ps[:], lp_row, neg2[:], start=False, stop=False,
                             skip_group_check=True)
            rstep[0] += 1
            nc.tensor.matmul(s_ps[:], logc_row[:], ones1[:], start=False,
                             stop=(rstep[0] == NRED), skip_group_check=True)
            nc.scalar.mul(s_col[:], s_ps[:], -0.5)   # ACT copy with scale
            gemms(wq3, x23, range(4, 6))       # quad k4-5
            gemms(wq3, x23, range(6, 8))       # quad k6-7 (stop)

            # ---- copy out (+ per-class const add) + DMA ----
            out_sb = sb.tile([128, BSH], F16, tag="osb")
            H2 = BSH // 2
            nc.vector.tensor_scalar_add(out_sb[:, 0:H2], pg[:, 0:H2], s_col[:])
            nc.sync.dma_start(out=out_d[:, 0:H2], in_=out_sb[:, 0:H2])
            nc.vector.tensor_scalar_add(out_sb[:, H2:BSH], pg[:, H2:BSH], s_col[:])
            nc.sync.dma_start(out=out_d[:, H2:BSH], in_=out_sb[:, H2:BSH])

    nc.compile()
    return nc


def get_nc():
    if "nc" not in _CACHE:
        _CACHE["nc"] = _build()
    return _CACHE["nc"]


def _pack_fmajor(aT):
    # (F=1024, n) f-major -> SBUF-packed (128, 8*n): cols k*n..(k+1)*n = rows
    # k*128..(k+1)*128
    Fdim, n = aT.shape
    k = Fdim // 128
    return np.ascontiguousarray(
        aT.reshape(k, 128, n).transpose(1, 0, 2).reshape(128, k * n)
    )


def make_in_maps(x, mu, log_var, log_pi):
    x16 = np.asarray(x, dtype=np.float16)
    mu16 = np.asarray(mu, dtype=np.float16)
    lv16 = np.asarray(log_var, dtype=np.float16)
    lp16 = np.asarray(log_pi, dtype=np.float16)

    xT = x16.T                                      # (1024, 2048)
    muT = mu16.T                                    # (1024, 256)
    lvT = lv16.T
    HW = WW // 2
    maps = []
    for core in range(NCORES):
        bi, ci = divmod(core, 2)
        cs = slice(ci * CSH, (ci + 1) * CSH)
        lvp = _pack_fmajor(lvT[:, cs])              # (128, 1024)
        mup = _pack_fmajor(muT[:, cs])
        wt = np.zeros((128, WP), np.float16)
        wt[:, 0:HW] = lvp[:, 0:HW]
        wt[:, HW:2 * HW] = mup[:, 0:HW]
        wt[:, 2 * HW:3 * HW] = lvp[:, HW:WW]
        wt[:, 3 * HW:4 * HW] = mup[:, HW:WW]
        wt[0, 4 * HW:4 * HW + CSH] = lp16[cs]
        maps.append({
            "wt": np.ascontiguousarray(wt),
            "xt": _pack_fmajor(xT[:, bi * BSH:(bi + 1) * BSH]),
        })
    return maps


def unpack_out(res):
    out = np.empty((B, C), dtype=np.float32)
    for core in range(NCORES):
        bi, ci = divmod(core, 2)
        o = res.results[core]["out"]                # (128c, 512b) fp16
        out[bi * BSH:(bi + 1) * BSH, ci * CSH:(ci + 1) * CSH] = o.T
    return out


def kernel(x, mu, log_var, log_pi):
    nc = get_nc()
    in_maps = make_in_maps(x, mu, log_var, log_pi)
    res = run_bass_kernel_spmd(nc, in_maps, list(range(NCORES)))
    return unpack_out(res)
